# revision 1
# baseline (speedup 1.0000x reference)
"""ConvCaps (shared 3x3 conv + dynamic routing) Trainium2 Bass kernel.

Sharding: data-parallel over batch B=8 -> 8 NeuronCores (1 image/core).
Per-core plan (all sizes hardcoded for B,Ci,Pi,Co,Po,K,H,W = 8,8,16,16,16,3,64,64):

  votes layout "D": 2 slabs [128=(co_s,po), (ci, S)] per spatial tile of S pixels.
  - conv: 3x3 conv as 2 matmuls per 512-chunk (K=128 taps0-7 x pi, K=16 tap8),
    im2col patches built by shifted SBUF->SBUF DMAs from a host-padded x.
  - iteration 0 softmax is uniform -> sj0 = conv(sum_ci x)/16 + bias, computed
    directly from a host-precomputed summed input (no routing pass needed).
  - b-side (coupling logits): W = V * vj (free-dim broadcast over ci), po-reduce
    on TensorE with two masked stationaries (OpoA/OpoB) accumulating both slabs
    into one [16, chunk] PSUM tile; exp on ScalarE doubles as the PSUM evac.
    bij never materialized raw: exp-product e2 = e1 * exp(upd) across iterations.
  - softmax over co: D = ones^T e on TensorE (replicated rows), reciprocal via
    fast Newton recip on VectorE, cij = e * (1/D).
  - sj-side: cij broadcast over po via TensorE (E0/E1 selection stationaries)
    into PSUM, U = V * cij_b on VectorE, sum over ci by TT add tree.
  - squash: r = sum_po sj^2 on TensorE (OpoA/OpoB), f = sqrt(r)/(1+r) on
    ScalarE+VectorE, fb = po-broadcast of f on TensorE, vj = sj * fb.
"""

import sys

sys.path.insert(0, "/opt/trn_rl_repo")

import numpy as np

import concourse.bacc as bacc
import concourse.mybir as mybir
import concourse.tile as tile
from concourse import bass_utils
from concourse import bass as bass_mod

F32 = mybir.dt.float32
ALU = mybir.AluOpType
ACTF = mybir.ActivationFunctionType

B, CI, PI, CO, PO, KK = 8, 8, 16, 16, 16, 3
H = W = 64
SP = H * W  # 4096 spatial positions
NCORES = 8
HP, WP = H + 2, W + 2  # padded 66x66

S = 512          # pixels per spatial tile (8 rows of 64)
NT = SP // S     # tiles per core
ROWS = S // W    # image rows per tile
CH = 512         # psum chunk (fp32 bank)
NCH = S // CH    # chunks per tile (=1 at S=512)


def _host_consts(conv_w: np.ndarray, biases: np.ndarray):
    """Precompute transposed weights / masks / bias vectors (fp32)."""
    cw = np.asarray(conv_w, np.float32)          # [Co*Po, Pi, 3, 3]
    # wT[(tap, pi), oc] ; tap = dh*3+dw
    wT = cw.transpose(2, 3, 1, 0).reshape(9 * PI, CO * PO).copy()  # [144, 256]
    wTa, wTb = wT[:128].copy(), wT[128:].copy()
    bv = np.asarray(biases, np.float32).reshape(CO, PO)  # [16,16]
    # biasv[(co_s,po), slab]
    biasv = np.zeros((128, 2), np.float32)
    for s in range(2):
        for cs in range(8):
            for po in range(PO):
                biasv[cs * 16 + po, s] = bv[s * 8 + cs, po]
    # E_s [16co, 128=(co_s,po)] : broadcast co over po for slab s
    E0 = np.zeros((16, 128), np.float32)
    E1 = np.zeros((16, 128), np.float32)
    for cs in range(8):
        E0[cs, cs * 16:(cs + 1) * 16] = 1.0
        E1[8 + cs, cs * 16:(cs + 1) * 16] = 1.0
    # Opo{A,B} [128=(co_s,po), 16] : po-sum, slab0 -> rows 0-7, slab1 -> rows 8-15
    OpA = np.zeros((128, 16), np.float32)
    OpB = np.zeros((128, 16), np.float32)
    for cs in range(8):
        OpA[cs * 16:(cs + 1) * 16, cs] = 1.0
        OpB[cs * 16:(cs + 1) * 16, 8 + cs] = 1.0
    ones16 = np.ones((16, 16), np.float32)
    return dict(wTa=wTa, wTb=wTb, wTa16=wTa / 16.0, wTb16=wTb / 16.0,
                biasv=biasv, E0=E0, E1=E1, OpA=OpA, OpB=OpB, ones16=ones16)


def _host_x(x: np.ndarray):
    """Per-core padded inputs: xpad [8cores][128=(ci,pi), 66*66], xsum [16, 66*66]."""
    xf = np.asarray(x, np.float32)  # [B, Ci, Pi, H, W]
    xpad = np.zeros((B, CI, PI, HP, WP), np.float32)
    xpad[:, :, :, 1:H + 1, 1:W + 1] = xf
    xs = xpad.sum(axis=1)  # [B, Pi, HP, WP]
    return (xpad.reshape(B, CI * PI, HP * WP),
            xs.reshape(B, PI, HP * WP))


def _build(nc, routings: int, reps: int):
    """Emit the per-core BIR program."""
    dram = {}
    for name, shape in [
        ("xpad", [CI * PI, HP * WP]), ("xsum", [PI, HP * WP]),
        ("wTa", [128, 256]), ("wTb", [16, 256]),
        ("wTa16", [128, 256]), ("wTb16", [16, 256]),
        ("biasv", [128, 2]), ("E0", [16, 128]), ("E1", [16, 128]),
        ("OpA", [128, 16]), ("OpB", [128, 16]), ("ones16", [16, 16]),
    ]:
        dram[name] = nc.dram_tensor(name, shape, F32, kind="ExternalInput")
    out_d = nc.dram_tensor("out", [CO * PO, SP], F32, kind="ExternalOutput")

    with tile.TileContext(nc) as tc:
        with (
            tc.tile_pool(name="const", bufs=1) as cpool,
            tc.tile_pool(name="vv", bufs=2) as vpool,
            tc.tile_pool(name="patch", bufs=2) as ppool,
            tc.tile_pool(name="wk", bufs=1) as wk,
            tc.tile_pool(name="wb", bufs=2) as wbpool,
            tc.tile_pool(name="ps_conv", bufs=2, space="PSUM") as ps_conv,
            tc.tile_pool(name="ps_cb", bufs=2, space="PSUM") as ps_cb,
            tc.tile_pool(name="ps_sm", bufs=2, space="PSUM") as ps_sm,
            tc.tile_pool(name="ps_b", bufs=2, space="PSUM") as ps_b,
        ):
            # ---- constants / inputs resident in SBUF ----
            xpad_sb = cpool.tile([CI * PI, HP * WP], F32, tag="xpad", name="xpad")
            xsum_sb = cpool.tile([PI, HP * WP], F32, tag="xsum", name="xsum")
            wts = {}
            for nm, shape in [("wTa", [128, 256]), ("wTb", [16, 256]),
                              ("wTa16", [128, 256]), ("wTb16", [16, 256]),
                              ("biasv", [128, 2]), ("E0", [16, 128]),
                              ("E1", [16, 128]), ("OpA", [128, 16]),
                              ("OpB", [128, 16]), ("ones16", [16, 16])]:
                wts[nm] = cpool.tile(shape, F32, tag=nm, name=nm)
                nc.sync.dma_start(wts[nm][:], dram[nm].ap())
            nc.sync.dma_start(xpad_sb[:], dram["xpad"].ap())
            nc.sync.dma_start(xsum_sb[:], dram["xsum"].ap())
            Es = [wts["E0"], wts["E1"]]
            wTa_s = lambda s: wts["wTa"][:, s * 128:(s + 1) * 128]
            wTb_s = lambda s: wts["wTb"][:, s * 128:(s + 1) * 128]
            wTa16_s = lambda s: wts["wTa16"][:, s * 128:(s + 1) * 128]
            wTb16_s = lambda s: wts["wTb16"][:, s * 128:(s + 1) * 128]
            bias_s = lambda s: wts["biasv"][:, s:s + 1]

            xpad3 = xpad_sb[:].rearrange("p (h w) -> p h w", h=HP)
            xsum3 = xsum_sb[:].rearrange("p (h w) -> p h w", h=HP)

            for _rep in range(reps):
                for t in range(NT):
                    r0 = t * ROWS  # first (unpadded) image row of this tile

                    # ---------------- conv -> votes V ----------------
                    V = [vpool.tile([128, CI * S], F32, tag=f"V{s}", name=f"V{s}") for s in range(2)]
                    for ci in range(CI):
                        pa = ppool.tile([128, S], F32, tag="pa", name="pa")
                        pb = ppool.tile([16, S], F32, tag="pb", name="pb")
                        src = xpad3[ci * PI:(ci + 1) * PI]
                        for tap in range(9):
                            dh, dw = tap // 3, tap % 3
                            sl = src[:, r0 + dh:r0 + dh + ROWS, dw:dw + W]
                            if tap < 8:
                                nc.sync.dma_start(
                                    pa[tap * 16:(tap + 1) * 16].rearrange(
                                        "p (r c) -> p r c", r=ROWS), sl)
                            else:
                                nc.sync.dma_start(
                                    pb[:].rearrange("p (r c) -> p r c", r=ROWS), sl)
                        for s in range(2):
                            for c in range(NCH):
                                cps = ps_conv.tile([128, CH], F32, tag="conv", name="conv")
                                nc.tensor.matmul(cps[:], wTa_s(s),
                                                 pa[:, c * CH:(c + 1) * CH],
                                                 start=True, stop=False)
                                nc.tensor.matmul(cps[:], wTb_s(s),
                                                 pb[:, c * CH:(c + 1) * CH],
                                                 start=False, stop=True)
                                dst = V[s][:, ci * S + c * CH: ci * S + (c + 1) * CH]
                                if ci % 2 == 0:
                                    nc.scalar.copy(dst, cps[:])
                                else:
                                    nc.vector.tensor_copy(dst, cps[:])

                    # ---------------- sj0 from summed input ----------------
                    pa2 = ppool.tile([128, S], F32, tag="pa2", name="pa2")
                    pb2 = ppool.tile([16, S], F32, tag="pb2", name="pb2")
                    for tap in range(9):
                        dh, dw = tap // 3, tap % 3
                        sl = xsum3[:, r0 + dh:r0 + dh + ROWS, dw:dw + W]
                        if tap < 8:
                            nc.sync.dma_start(
                                pa2[tap * 16:(tap + 1) * 16].rearrange(
                                    "p (r c) -> p r c", r=ROWS), sl)
                        else:
                            nc.sync.dma_start(
                                pb2[:].rearrange("p (r c) -> p r c", r=ROWS), sl)

                    sjs = [wk.tile([128, S], F32, tag=f"sjs{s}", name=f"sjs{s}") for s in range(2)]
                    vjs = [wk.tile([128, S], F32, tag=f"vjs{s}", name=f"vjs{s}") for s in range(2)]
                    e_run = wk.tile([16, CI * S], F32, tag="e_run", name="e_run")
                    cij = wk.tile([16, CI * S], F32, tag="cij", name="cij")
                    sq = [wk.tile([128, S], F32, tag=f"sq{s}", name=f"sq{s}") for s in range(2)]
                    f_t = wk.tile([16, S], F32, tag="f_t", name="f_t")
                    fw = wk.tile([16, S], F32, tag="fw", name="fw")
                    fr = wk.tile([16, S], F32, tag="fr", name="fr")

                    for s in range(2):
                        for c in range(NCH):
                            cps = ps_conv.tile([128, CH], F32, tag="conv", name="conv")
                            nc.tensor.matmul(cps[:], wTa16_s(s),
                                             pa2[:, c * CH:(c + 1) * CH],
                                             start=True, stop=False)
                            nc.tensor.matmul(cps[:], wTb16_s(s),
                                             pb2[:, c * CH:(c + 1) * CH],
                                             start=False, stop=True)
                            nc.vector.tensor_add(
                                sjs[s][:, c * CH:(c + 1) * CH], cps[:],
                                bias_s(s).to_broadcast((128, CH)))

                    # ---------------- routing iterations ----------------
                    for it in range(routings):
                        if it > 0:
                            # sj-side: U = V * cij_b ; sum over ci ; + bias
                            for s in range(2):
                                U = wk.tile([128, CI * S], F32, tag="U", name="U")
                                for ci in range(CI):
                                    for c in range(NCH):
                                        cb = ps_cb.tile([128, CH], F32, tag="cb", name="cb")
                                        nc.tensor.matmul(
                                            cb[:], Es[s],
                                            cij[:, ci * S + c * CH: ci * S + (c + 1) * CH],
                                            start=True, stop=True)
                                        o = ci * S + c * CH
                                        nc.vector.tensor_mul(
                                            U[:, o:o + CH],
                                            V[s][:, o:o + CH], cb[:])
                                T1 = wk.tile([128, 4 * S], F32, tag="T1", name="T1")
                                T2 = wk.tile([128, 2 * S], F32, tag="T2", name="T2")
                                nc.vector.tensor_add(T1[:], U[:, :4 * S], U[:, 4 * S:])
                                nc.vector.tensor_add(T2[:], T1[:, :2 * S], T1[:, 2 * S:])
                                nc.vector.scalar_tensor_tensor(
                                    out=sjs[s][:], in0=T2[:, :S], scalar=bias_s(s),
                                    in1=T2[:, S:], op0=ALU.add, op1=ALU.add)

                        # squash: vj = sj * sqrt(r)/(1+r), r = sum_po sj^2
                        for s in range(2):
                            nc.scalar.activation(sq[s][:], sjs[s][:], ACTF.Square)
                        for c in range(NCH):
                            rps = ps_sm.tile([128, CH], F32, tag="sm", name="rps")[:16]
                            nc.tensor.matmul(rps[:], wts["OpA"],
                                             sq[0][:, c * CH:(c + 1) * CH],
                                             start=True, stop=False)
                            nc.tensor.matmul(rps[:], wts["OpB"],
                                             sq[1][:, c * CH:(c + 1) * CH],
                                             start=False, stop=True)
                            nc.scalar.activation(f_t[:, c * CH:(c + 1) * CH],
                                                 rps[:], ACTF.Sqrt)
                            nc.scalar.add(fw[:, c * CH:(c + 1) * CH], rps[:], 1.0)
                        nc.vector.reciprocal_approx_fast(fr[:], fw[:])
                        nc.vector.tensor_mul(f_t[:], f_t[:], fr[:])
                        for s in range(2):
                            for c in range(NCH):
                                fb = ps_sm.tile([128, CH], F32, tag="sm", name="fbps")
                                nc.tensor.matmul(fb[:], Es[s],
                                                 f_t[:, c * CH:(c + 1) * CH],
                                                 start=True, stop=True)
                                nc.vector.tensor_mul(
                                    vjs[s][:, c * CH:(c + 1) * CH],
                                    sjs[s][:, c * CH:(c + 1) * CH], fb[:])

                        if it == routings - 1:
                            for s in range(2):
                                nc.sync.dma_start(
                                    out_d.ap()[s * 128:(s + 1) * 128,
                                               t * S:(t + 1) * S], vjs[s][:])
                            continue

                        # b-side: upd = sum_po V*vj ; e_run *= exp(upd)
                        for ci in range(CI):
                            for c in range(NCH):
                                bp = ps_b.tile([16, CH], F32, tag="bij", name="bij")
                                for s in range(2):
                                    wb = wbpool.tile([128, CH], F32, tag="wb", name="wb")
                                    o = ci * S + c * CH
                                    nc.vector.tensor_mul(
                                        wb[:], V[s][:, o:o + CH],
                                        vjs[s][:, c * CH:(c + 1) * CH])
                                    nc.tensor.matmul(bp[:],
                                                     wts["OpA"] if s == 0 else wts["OpB"],
                                                     wb[:], start=(s == 0),
                                                     stop=(s == 1))
                                o = ci * S + c * CH
                                if it == 0:
                                    nc.scalar.activation(e_run[:, o:o + CH],
                                                         bp[:], ACTF.Exp)
                                else:
                                    ex = wbpool.tile([16, CH], F32, tag="ex", name="ex")
                                    nc.scalar.activation(ex[:], bp[:], ACTF.Exp)
                                    nc.vector.tensor_mul(
                                        e_run[:, o:o + CH],
                                        e_run[:, o:o + CH], ex[:])

                        # softmax denom over co and normalize
                        for g in range(CI * S // CH):
                            dp = ps_sm.tile([128, CH], F32, tag="sm", name="dps")[:16]
                            nc.tensor.matmul(dp[:], wts["ones16"],
                                             e_run[:, g * CH:(g + 1) * CH],
                                             start=True, stop=True)
                            drc = wk.tile([16, CH], F32, tag="drc", name="drc")
                            nc.vector.reciprocal_approx_fast(drc[:], dp[:])
                            nc.vector.tensor_mul(cij[:, g * CH:(g + 1) * CH],
                                                 e_run[:, g * CH:(g + 1) * CH],
                                                 drc[:])

    return dram, out_d


_CACHE = {}


def _get_compiled(routings: int, reps: int):
    key = (routings, reps)
    if key not in _CACHE:
        nc = bacc.Bacc("TRN2", target_bir_lowering=False, debug=False,
                       num_devices=NCORES)
        _build(nc, routings, reps)
        nc.compile()
        _CACHE[key] = nc
    return _CACHE[key]


def _in_maps(x, conv_w, biases):
    consts = _host_consts(conv_w, biases)
    xpad, xsum = _host_x(x)
    maps = []
    for b in range(NCORES):
        m = {k: np.ascontiguousarray(v) for k, v in consts.items()}
        m["xpad"] = np.ascontiguousarray(xpad[b])
        m["xsum"] = np.ascontiguousarray(xsum[b])
        maps.append(m)
    return maps


def _run(nc, in_maps):
    res = bass_utils.run_bass_kernel_spmd(nc, in_maps,
                                          core_ids=list(range(NCORES)))
    out = np.stack([res.results[b]["out"] for b in range(NCORES)], axis=0)
    return out.reshape(B, CO, PO, H, W)


def kernel(x, conv_w, biases, routings):
    routings = int(routings)
    nc = _get_compiled(routings, reps=1)
    return _run(nc, _in_maps(x, conv_w, biases))


def measure_hw_time_ns(inputs, reps=4, samples=3):
    """Wall-clock repetition-delta: per-iteration HW time, dispatch cancelled."""
    import time
    routings = int(inputs["routings"])
    maps = _in_maps(inputs["x"], inputs["conv_w"], inputs["biases"])
    nc1 = _get_compiled(routings, reps=1)
    ncR = _get_compiled(routings, reps=reps)

    def timed(nc):
        best = float("inf")
        for _ in range(samples):
            t0 = time.perf_counter()
            bass_utils.run_bass_kernel_spmd(nc, maps, core_ids=list(range(NCORES)))
            best = min(best, time.perf_counter() - t0)
        return best

    timed(nc1); timed(ncR)  # warm both (compile + jit caches)
    t1 = timed(nc1)
    tR = timed(ncR)
    print(f"  raw wall: 1rep {t1*1e3:.1f} ms, {reps}rep {tR*1e3:.1f} ms")
    return int((tR - t1) / (reps - 1) * 1e9)



# revision 4
# speedup vs baseline: 301.9291x; 301.9291x over previous
"""ConvCaps (shared 3x3 conv + dynamic routing) Trainium2 Bass kernel.

Sharding: data-parallel over batch B=8 -> 8 NeuronCores (1 image/core).
Per-core plan (all sizes hardcoded for B,Ci,Pi,Co,Po,K,H,W = 8,8,16,16,16,3,64,64):

  votes layout "D": 2 slabs [128=(co_s,po), (ci, S)] per spatial tile of S pixels.
  - conv: 3x3 conv as 2 matmuls per 512-chunk (K=128 taps0-7 x pi, K=16 tap8),
    im2col patches built by shifted SBUF->SBUF DMAs from a host-padded x.
  - iteration 0 softmax is uniform -> sj0 = conv(sum_ci x)/16 + bias, computed
    directly from a host-precomputed summed input (no routing pass needed).
  - b-side (coupling logits): W = V * vj (free-dim broadcast over ci), po-reduce
    on TensorE with two masked stationaries (OpoA/OpoB) accumulating both slabs
    into one [16, chunk] PSUM tile; exp on ScalarE doubles as the PSUM evac.
    bij never materialized raw: exp-product e2 = e1 * exp(upd) across iterations.
  - softmax over co: D = ones^T e on TensorE (replicated rows), reciprocal via
    fast Newton recip on VectorE, cij = e * (1/D).
  - sj-side: cij broadcast over po via TensorE (E0/E1 selection stationaries)
    into PSUM, U = V * cij_b on VectorE, sum over ci by TT add tree.
  - squash: r = sum_po sj^2 on TensorE (OpoA/OpoB), f = sqrt(r)/(1+r) on
    ScalarE+VectorE, fb = po-broadcast of f on TensorE, vj = sj * fb.
"""

import os
import sys

sys.path.insert(0, "/opt/trn_rl_repo")

import numpy as np

import concourse.bacc as bacc
import concourse.mybir as mybir
import concourse.tile as tile
from concourse import bass_utils
from concourse import bass as bass_mod

F32 = mybir.dt.float32
ALU = mybir.AluOpType
ACTF = mybir.ActivationFunctionType

B, CI, PI, CO, PO, KK = 8, 8, 16, 16, 16, 3
H = W = 64
SP = H * W  # 4096 spatial positions
NCORES = 8
HP, WP = H + 2, W + 2  # padded 66x66

S = 512          # pixels per spatial tile (8 rows of 64)
NT = SP // S     # tiles per core
ROWS = S // W    # image rows per tile
CH = 512         # psum chunk (fp32 bank)
NCH = S // CH    # chunks per tile (=1 at S=512)


def _host_consts(conv_w: np.ndarray, biases: np.ndarray):
    """Precompute transposed weights / masks / bias vectors (fp32)."""
    cw = np.asarray(conv_w, np.float32)          # [Co*Po, Pi, 3, 3]
    # wT[(tap, pi), oc] ; tap = dh*3+dw
    wT = cw.transpose(2, 3, 1, 0).reshape(9 * PI, CO * PO).copy()  # [144, 256]
    wTa, wTb = wT[:128].copy(), wT[128:].copy()
    bv = np.asarray(biases, np.float32).reshape(CO, PO)  # [16,16]
    # biasv[(co_s,po), slab]
    biasv = np.zeros((128, 2), np.float32)
    for s in range(2):
        for cs in range(8):
            for po in range(PO):
                biasv[cs * 16 + po, s] = bv[s * 8 + cs, po]
    # E_s [16co, 128=(co_s,po)] : broadcast co over po for slab s
    E0 = np.zeros((16, 128), np.float32)
    E1 = np.zeros((16, 128), np.float32)
    for cs in range(8):
        E0[cs, cs * 16:(cs + 1) * 16] = 1.0
        E1[8 + cs, cs * 16:(cs + 1) * 16] = 1.0
    # Opo{A,B} [128=(co_s,po), 16] : po-sum, slab0 -> rows 0-7, slab1 -> rows 8-15
    OpA = np.zeros((128, 16), np.float32)
    OpB = np.zeros((128, 16), np.float32)
    for cs in range(8):
        OpA[cs * 16:(cs + 1) * 16, cs] = 1.0
        OpB[cs * 16:(cs + 1) * 16, 8 + cs] = 1.0
    ones16 = np.ones((16, 16), np.float32)
    return dict(wTa=wTa, wTb=wTb, wTa16=wTa / 16.0, wTb16=wTb / 16.0,
                biasv=biasv, E0=E0, E1=E1, OpA=OpA, OpB=OpB, ones16=ones16)


def _host_x(x: np.ndarray):
    """Per-core padded inputs: xpad [8cores][128=(ci,pi), 66*66], xsum [16, 66*66]."""
    xf = np.asarray(x, np.float32)  # [B, Ci, Pi, H, W]
    xpad = np.zeros((B, CI, PI, HP, WP), np.float32)
    xpad[:, :, :, 1:H + 1, 1:W + 1] = xf
    xs = xpad.sum(axis=1)  # [B, Pi, HP, WP]
    return (xpad.reshape(B, CI * PI, HP * WP),
            xs.reshape(B, PI, HP * WP))


def _build(nc, routings: int, reps: int):
    """Emit the per-core BIR program."""
    dram = {}
    for name, shape in [
        ("xpad", [CI * PI, HP * WP]), ("xsum", [PI, HP * WP]),
        ("wTa", [128, 256]), ("wTb", [16, 256]),
        ("wTa16", [128, 256]), ("wTb16", [16, 256]),
        ("biasv", [128, 2]), ("E0", [16, 128]), ("E1", [16, 128]),
        ("OpA", [128, 16]), ("OpB", [128, 16]), ("ones16", [16, 16]),
    ]:
        dram[name] = nc.dram_tensor(name, shape, F32, kind="ExternalInput")
    out_d = nc.dram_tensor("out", [CO * PO, SP], F32, kind="ExternalOutput")

    with tile.TileContext(nc) as tc:
        with (
            tc.tile_pool(name="const", bufs=1) as cpool,
            tc.tile_pool(name="vv", bufs=2) as vpool,
            tc.tile_pool(name="patch", bufs=2) as ppool,
            tc.tile_pool(name="wk", bufs=1) as wk,
            tc.tile_pool(name="wb", bufs=2) as wbpool,
            tc.tile_pool(name="ps_conv", bufs=2, space="PSUM") as ps_conv,
            tc.tile_pool(name="ps_cb", bufs=2, space="PSUM") as ps_cb,
            tc.tile_pool(name="ps_sm", bufs=2, space="PSUM") as ps_sm,
            tc.tile_pool(name="ps_b", bufs=2, space="PSUM") as ps_b,
        ):
            # ---- constants / inputs resident in SBUF ----
            xpad_sb = cpool.tile([CI * PI, HP * WP], F32, tag="xpad", name="xpad")
            xsum_sb = cpool.tile([PI, HP * WP], F32, tag="xsum", name="xsum")
            wts = {}
            for nm, shape in [("wTa", [128, 256]), ("wTb", [16, 256]),
                              ("wTa16", [128, 256]), ("wTb16", [16, 256]),
                              ("biasv", [128, 2]), ("E0", [16, 128]),
                              ("E1", [16, 128]), ("OpA", [128, 16]),
                              ("OpB", [128, 16]), ("ones16", [16, 16])]:
                wts[nm] = cpool.tile(shape, F32, tag=nm, name=nm)
                nc.sync.dma_start(wts[nm][:], dram[nm].ap())
            nc.sync.dma_start(xpad_sb[:], dram["xpad"].ap())
            nc.sync.dma_start(xsum_sb[:], dram["xsum"].ap())
            Es = [wts["E0"], wts["E1"]]
            wTa_s = lambda s: wts["wTa"][:, s * 128:(s + 1) * 128]
            wTb_s = lambda s: wts["wTb"][:, s * 128:(s + 1) * 128]
            wTa16_s = lambda s: wts["wTa16"][:, s * 128:(s + 1) * 128]
            wTb16_s = lambda s: wts["wTb16"][:, s * 128:(s + 1) * 128]
            bias_s = lambda s: wts["biasv"][:, s:s + 1]

            xpad3 = xpad_sb[:].rearrange("p (h w) -> p h w", h=HP)
            xsum3 = xsum_sb[:].rearrange("p (h w) -> p h w", h=HP)

            for _rep in range(reps):
                for t in range(NT):
                    r0 = t * ROWS  # first (unpadded) image row of this tile

                    # ---------------- conv -> votes V ----------------
                    V = [vpool.tile([128, CI * S], F32, tag=f"V{s}", name=f"V{s}") for s in range(2)]
                    for ci in range(CI):
                        pa = ppool.tile([128, S], F32, tag="pa", name="pa")
                        pb = ppool.tile([16, S], F32, tag="pb", name="pb")
                        src = xpad3[ci * PI:(ci + 1) * PI]
                        for tap in range(9):
                            dh, dw = tap // 3, tap % 3
                            sl = src[:, r0 + dh:r0 + dh + ROWS, dw:dw + W]
                            if tap < 8:
                                nc.sync.dma_start(
                                    pa[tap * 16:(tap + 1) * 16].rearrange(
                                        "p (r c) -> p r c", r=ROWS), sl)
                            else:
                                nc.sync.dma_start(
                                    pb[:].rearrange("p (r c) -> p r c", r=ROWS), sl)
                        for s in range(2):
                            for c in range(NCH):
                                cps = ps_conv.tile([128, CH], F32, tag="conv", name="conv")
                                nc.tensor.matmul(cps[:], wTa_s(s),
                                                 pa[:, c * CH:(c + 1) * CH],
                                                 start=True, stop=False)
                                nc.tensor.matmul(cps[:], wTb_s(s),
                                                 pb[:, c * CH:(c + 1) * CH],
                                                 start=False, stop=True)
                                dst = V[s][:, ci * S + c * CH: ci * S + (c + 1) * CH]
                                if ci % 2 == 0:
                                    nc.scalar.copy(dst, cps[:])
                                else:
                                    nc.vector.tensor_copy(dst, cps[:])

                    # ---------------- sj0 from summed input ----------------
                    pa2 = ppool.tile([128, S], F32, tag="pa2", name="pa2")
                    pb2 = ppool.tile([16, S], F32, tag="pb2", name="pb2")
                    for tap in range(9):
                        dh, dw = tap // 3, tap % 3
                        sl = xsum3[:, r0 + dh:r0 + dh + ROWS, dw:dw + W]
                        if tap < 8:
                            nc.sync.dma_start(
                                pa2[tap * 16:(tap + 1) * 16].rearrange(
                                    "p (r c) -> p r c", r=ROWS), sl)
                        else:
                            nc.sync.dma_start(
                                pb2[:].rearrange("p (r c) -> p r c", r=ROWS), sl)

                    sjs = [wk.tile([128, S], F32, tag=f"sjs{s}", name=f"sjs{s}") for s in range(2)]
                    vjs = [wk.tile([128, S], F32, tag=f"vjs{s}", name=f"vjs{s}") for s in range(2)]
                    e_run = wk.tile([16, CI * S], F32, tag="e_run", name="e_run")
                    cij = wk.tile([16, CI * S], F32, tag="cij", name="cij")
                    sq = [wk.tile([128, S], F32, tag=f"sq{s}", name=f"sq{s}") for s in range(2)]
                    f_t = wk.tile([16, S], F32, tag="f_t", name="f_t")
                    fw = wk.tile([16, S], F32, tag="fw", name="fw")
                    fr = wk.tile([16, S], F32, tag="fr", name="fr")

                    for s in range(2):
                        for c in range(NCH):
                            cps = ps_conv.tile([128, CH], F32, tag="conv", name="conv")
                            nc.tensor.matmul(cps[:], wTa16_s(s),
                                             pa2[:, c * CH:(c + 1) * CH],
                                             start=True, stop=False)
                            nc.tensor.matmul(cps[:], wTb16_s(s),
                                             pb2[:, c * CH:(c + 1) * CH],
                                             start=False, stop=True)
                            nc.vector.tensor_add(
                                sjs[s][:, c * CH:(c + 1) * CH], cps[:],
                                bias_s(s).to_broadcast((128, CH)))

                    # ---------------- routing iterations ----------------
                    for it in range(routings):
                        if it > 0:
                            # sj-side: U = V * cij_b ; sum over ci ; + bias
                            for s in range(2):
                                U = wk.tile([128, CI * S], F32, tag="U", name="U")
                                for ci in range(CI):
                                    for c in range(NCH):
                                        cb = ps_cb.tile([128, CH], F32, tag="cb", name="cb")
                                        nc.tensor.matmul(
                                            cb[:], Es[s],
                                            cij[:, ci * S + c * CH: ci * S + (c + 1) * CH],
                                            start=True, stop=True)
                                        o = ci * S + c * CH
                                        nc.vector.tensor_mul(
                                            U[:, o:o + CH],
                                            V[s][:, o:o + CH], cb[:])
                                T1 = wk.tile([128, 4 * S], F32, tag="T1", name="T1")
                                T2 = wk.tile([128, 2 * S], F32, tag="T2", name="T2")
                                nc.vector.tensor_add(T1[:], U[:, :4 * S], U[:, 4 * S:])
                                nc.vector.tensor_add(T2[:], T1[:, :2 * S], T1[:, 2 * S:])
                                nc.vector.scalar_tensor_tensor(
                                    out=sjs[s][:], in0=T2[:, :S], scalar=bias_s(s),
                                    in1=T2[:, S:], op0=ALU.add, op1=ALU.add)

                        # squash: vj = sj * sqrt(r)/(1+r), r = sum_po sj^2
                        for s in range(2):
                            nc.scalar.activation(sq[s][:], sjs[s][:], ACTF.Square)
                        for c in range(NCH):
                            rps = ps_sm.tile([128, CH], F32, tag="sm", name="rps")[:16]
                            nc.tensor.matmul(rps[:], wts["OpA"],
                                             sq[0][:, c * CH:(c + 1) * CH],
                                             start=True, stop=False)
                            nc.tensor.matmul(rps[:], wts["OpB"],
                                             sq[1][:, c * CH:(c + 1) * CH],
                                             start=False, stop=True)
                            nc.scalar.activation(f_t[:, c * CH:(c + 1) * CH],
                                                 rps[:], ACTF.Sqrt)
                            nc.scalar.add(fw[:, c * CH:(c + 1) * CH], rps[:], 1.0)
                        nc.vector.reciprocal_approx_fast(fr[:], fw[:])
                        nc.vector.tensor_mul(f_t[:], f_t[:], fr[:])
                        for s in range(2):
                            for c in range(NCH):
                                fb = ps_sm.tile([128, CH], F32, tag="sm", name="fbps")
                                nc.tensor.matmul(fb[:], Es[s],
                                                 f_t[:, c * CH:(c + 1) * CH],
                                                 start=True, stop=True)
                                nc.vector.tensor_mul(
                                    vjs[s][:, c * CH:(c + 1) * CH],
                                    sjs[s][:, c * CH:(c + 1) * CH], fb[:])

                        if it == routings - 1:
                            for s in range(2):
                                nc.sync.dma_start(
                                    out_d.ap()[s * 128:(s + 1) * 128,
                                               t * S:(t + 1) * S], vjs[s][:])
                            continue

                        # b-side: upd = sum_po V*vj ; e_run *= exp(upd)
                        for ci in range(CI):
                            for c in range(NCH):
                                bp = ps_b.tile([16, CH], F32, tag="bij", name="bij")
                                for s in range(2):
                                    wb = wbpool.tile([128, CH], F32, tag="wb", name="wb")
                                    o = ci * S + c * CH
                                    nc.vector.tensor_mul(
                                        wb[:], V[s][:, o:o + CH],
                                        vjs[s][:, c * CH:(c + 1) * CH])
                                    nc.tensor.matmul(bp[:],
                                                     wts["OpA"] if s == 0 else wts["OpB"],
                                                     wb[:], start=(s == 0),
                                                     stop=(s == 1))
                                o = ci * S + c * CH
                                if it == 0:
                                    nc.scalar.activation(e_run[:, o:o + CH],
                                                         bp[:], ACTF.Exp)
                                else:
                                    ex = wbpool.tile([16, CH], F32, tag="ex", name="ex")
                                    nc.scalar.activation(ex[:], bp[:], ACTF.Exp)
                                    nc.vector.tensor_mul(
                                        e_run[:, o:o + CH],
                                        e_run[:, o:o + CH], ex[:])

                        # softmax denom over co and normalize
                        for g in range(CI * S // CH):
                            dp = ps_sm.tile([128, CH], F32, tag="sm", name="dps")[:16]
                            nc.tensor.matmul(dp[:], wts["ones16"],
                                             e_run[:, g * CH:(g + 1) * CH],
                                             start=True, stop=True)
                            drc = wk.tile([16, CH], F32, tag="drc", name="drc")
                            nc.vector.reciprocal_approx_fast(drc[:], dp[:])
                            nc.vector.tensor_mul(cij[:, g * CH:(g + 1) * CH],
                                                 e_run[:, g * CH:(g + 1) * CH],
                                                 drc[:])

    return dram, out_d


_CACHE = {}


def _get_compiled(routings: int, reps: int):
    key = (routings, reps)
    if key not in _CACHE:
        nc = bacc.Bacc("TRN2", target_bir_lowering=False, debug=False,
                       num_devices=NCORES)
        _build(nc, routings, reps)
        nc.compile()
        _CACHE[key] = nc
    return _CACHE[key]


_RUNNERS = {}


def _get_runner(nc):
    """Persistent jitted executor for a compiled Bass module.

    run_bass_kernel_spmd builds a fresh jax.jit closure per call, so every
    invocation re-runs BIR verification + neuronx-cc (seconds). Caching the
    jitted callable makes repeat executions pure upload+execute+download.
    """
    key = id(nc)
    if key in _RUNNERS:
        return _RUNNERS[key]

    import jax
    import numpy as _np
    from jax.experimental.shard_map import shard_map
    from jax.sharding import Mesh, PartitionSpec
    from concourse import bass2jax
    from concourse import mybir as _mybir

    bass2jax.install_neuronx_cc_hook()

    partition_name = (nc.partition_id_tensor.name
                      if nc.partition_id_tensor else None)
    in_names, out_names, out_avals, zero_shapes = [], [], [], []
    for alloc in nc.m.functions[0].allocations:
        if not isinstance(alloc, _mybir.MemoryLocationSet):
            continue
        name = alloc.memorylocations[0].name
        if alloc.kind == "ExternalInput":
            if name != partition_name:
                in_names.append(name)
        elif alloc.kind == "ExternalOutput":
            shape = tuple(alloc.tensor_shape)
            dtype = _mybir.dt.np(alloc.dtype)
            out_names.append(name)
            out_avals.append(jax.core.ShapedArray(shape, dtype))
            zero_shapes.append((shape, dtype))
    n_params = len(in_names)
    n_outs = len(out_avals)
    all_in_names = list(in_names) + list(out_names)
    if partition_name is not None:
        all_in_names.append(partition_name)
    donate = tuple(range(n_params, n_params + n_outs))

    def _body(*args):
        operands = list(args)
        if partition_name is not None:
            operands.append(bass2jax.partition_id_tensor())
        outs = bass2jax._bass_exec_p.bind(
            *operands,
            out_avals=tuple(out_avals),
            in_names=tuple(all_in_names),
            out_names=tuple(out_names),
            lowering_input_output_aliases=(),
            sim_require_finite=True,
            sim_require_nnan=True,
            nc=nc,
        )
        return tuple(outs)

    devices = jax.devices()[:NCORES]
    mesh = Mesh(_np.asarray(devices), ("core",))
    in_specs = (PartitionSpec("core"),) * (n_params + n_outs)
    out_specs = (PartitionSpec("core"),) * n_outs
    sharded = jax.jit(
        shard_map(_body, mesh=mesh, in_specs=in_specs, out_specs=out_specs,
                  check_rep=False),
        donate_argnums=donate, keep_unused=True,
    )

    def run(in_maps, as_numpy=True):
        concat_in = [
            np.concatenate([np.asarray(in_maps[c][nm])
                            for c in range(NCORES)], axis=0)
            for nm in in_names
        ]
        concat_zeros = [np.zeros((NCORES * s[0], *s[1:]), d)
                        for s, d in zero_shapes]
        out_arrs = sharded(*concat_in, *concat_zeros)
        if not as_numpy:
            jax.block_until_ready(out_arrs)
            return None
        return [
            {nm: np.asarray(out_arrs[i]).reshape(
                NCORES, *out_avals[i].shape)[c]
             for i, nm in enumerate(out_names)}
            for c in range(NCORES)
        ]

    _RUNNERS[key] = run
    return run


def _axon_profile_lib():
    import ctypes
    lib = ctypes.CDLL("/opt/axon/libaxon_pjrt.so")
    if not hasattr(lib, "axon_start_nrt_profile"):
        return None
    lib.axon_start_nrt_profile.argtypes = [ctypes.POINTER(ctypes.c_int64),
                                           ctypes.c_size_t]
    lib.axon_start_nrt_profile.restype = ctypes.c_int64
    lib.axon_stop_nrt_profile.argtypes = [ctypes.c_char_p]
    lib.axon_stop_nrt_profile.restype = ctypes.c_int64
    return lib


def _in_maps(x, conv_w, biases):
    consts = _host_consts(conv_w, biases)
    xpad, xsum = _host_x(x)
    maps = []
    for b in range(NCORES):
        m = {k: np.ascontiguousarray(v) for k, v in consts.items()}
        m["xpad"] = np.ascontiguousarray(xpad[b])
        m["xsum"] = np.ascontiguousarray(xsum[b])
        maps.append(m)
    return maps


def _run(nc, in_maps):
    res = _get_runner(nc)(in_maps)
    out = np.stack([res[b]["out"] for b in range(NCORES)], axis=0)
    return out.reshape(B, CO, PO, H, W)


def kernel(x, conv_w, biases, routings):
    routings = int(routings)
    nc = _get_compiled(routings, reps=1)
    return _run(nc, _in_maps(x, conv_w, biases))


def _parse_ntff_total_times(outdir):
    """neuron-profile each .ntff in outdir -> {(exec_id, dev): total_time_s}."""
    import glob as _glob, json, re, subprocess
    from concurrent.futures import ThreadPoolExecutor

    neffs = _glob.glob(os.path.join(outdir, "*.neff"))
    ntffs = sorted(_glob.glob(os.path.join(outdir, "*.ntff")))
    if not neffs or not ntffs:
        return {}
    neff = neffs[0]

    def one(path):
        m = re.search(r"device(\d+)-execution-(\d+)\.ntff$", path)
        if not m:
            return None
        dev, ex = int(m.group(1)), int(m.group(2))
        try:
            p = subprocess.run(
                ["neuron-profile", "view", "-n", neff, "-s", path,
                 "--output-format", "summary-json"],
                capture_output=True, text=True, timeout=600)
            line = p.stdout.strip().splitlines()[-1]
            summ = json.loads(line)
            total = next(iter(summ.values()))["total_time"]
            return (ex, dev), float(total)
        except Exception:
            return None

    with ThreadPoolExecutor(max_workers=8) as tp:
        results = list(tp.map(one, ntffs))
    return {k: v for r in results if r for k, v in [r]}


def measure_hw_time_ns(inputs, reps=1, samples=3):
    """Real device execution time via NRT/NTFF profiling of the jitted NEFF.

    Profiles `samples` executions; per execution takes the max total_time
    across the 8 cores (completion = slowest core), returns the min over
    executions in ns. Falls back to wall-clock repetition-delta if the
    profiling hook is unavailable.
    """
    import tempfile, time
    import jax
    routings = int(inputs["routings"])
    maps = _in_maps(inputs["x"], inputs["conv_w"], inputs["biases"])
    nc = _get_compiled(routings, reps=reps)
    run = _get_runner(nc)
    run(maps, as_numpy=False)  # compile + warm
    run(maps, as_numpy=False)

    lib = _axon_profile_lib()
    if lib is not None:
        outdir = tempfile.mkdtemp(prefix="ntffprof_")
        jax.devices()
        rc = lib.axon_start_nrt_profile(None, 0)
        if rc == 0:
            for _ in range(samples):
                run(maps, as_numpy=False)
            n = lib.axon_stop_nrt_profile(outdir.encode())
            if n > 0:
                times = _parse_ntff_total_times(outdir)
                if times:
                    by_exec = {}
                    for (ex, dev), t in times.items():
                        by_exec.setdefault(ex, []).append(t)
                    per_exec = [max(v) for v in by_exec.values()]
                    return int(min(per_exec) / reps * 1e9)

    # Fallback: wall-clock delta between reps=1 and reps=R programs.
    ncR = _get_compiled(routings, reps=4)
    runR = _get_runner(ncR)
    runR(maps, as_numpy=False)

    def timed(fn):
        best = float("inf")
        for _ in range(samples):
            t0 = time.perf_counter()
            fn(maps, as_numpy=False)
            best = min(best, time.perf_counter() - t0)
        return best

    t1 = timed(run)
    tR = timed(runR)
    return int((tR - t1) / 3 * 1e9)

